# revision 4
# baseline (speedup 1.0000x reference)
"""GQA attention (BagleyAttention) on 8 Trainium2 NeuronCores — fused v2.

Tensor-parallel over kv-head groups: core c owns kv head c and query heads
[4c, 4c+4). Each core computes its heads' attention and a partial output
projection [S, D] in fp16; the host sums the 8 partials.

Single fused pipeline (no phase barriers): projection of s-block sb
overlaps attention of q-block t=sb-1, which overlaps the output
projection of q-block t-1. RoPE rotate-half uses partition-shifted DVE
reads of the projection PSUM (no DMA swap). Softmax normalizer uses a
[1,512] reciprocal + gpsimd partition_broadcast.

PSUM budget (8 banks): proj pool 3 (two passes of 3 accumulators per
s-block), hot pool 3 (score tiles + outproj tiles + V-transpose tiles),
accq 2 (PV + exp-sum accumulators, bank-aligned).
"""

import math
import sys

sys.path.insert(0, "/opt/trn_rl_repo")

import numpy as np

B, S, D = 1, 2048, 4096
H, KV, Dh = 32, 8, 128
G = H // KV            # query heads per kv head (= per core)
EH = G * Dh            # per-core q projection width (512)
N_CORES = 8

SB = 512               # s-block width
N_SB = S // SB         # 4
N_DC = D // 128        # 32 d-chunks
N_NB = D // SB         # 8 output d-blocks

EXP_BIAS = 9.5         # exp(s - EXP_BIAS); cancels in softmax normalization

_cache = {}


def _build():
    import concourse.mybir as mybir
    import concourse.tile as tile
    from concourse import bacc
    from concourse.masks import make_identity

    dt = mybir.dt
    f32, f16 = dt.float32, dt.float16
    AF = mybir.ActivationFunctionType

    nc = bacc.Bacc("TRN2", target_bir_lowering=False, debug=False)

    xT = nc.dram_tensor("xT", [D, S], f16, kind="ExternalInput").ap()
    wqT = nc.dram_tensor("wqT", [D, EH], f16, kind="ExternalInput").ap()
    wkT = nc.dram_tensor("wkT", [D, Dh], f16, kind="ExternalInput").ap()
    wvT = nc.dram_tensor("wvT", [D, Dh], f16, kind="ExternalInput").ap()
    woT = nc.dram_tensor("woT", [EH, D], f16, kind="ExternalInput").ap()
    cosT = nc.dram_tensor("cosT", [Dh, S], f16, kind="ExternalInput").ap()
    sinmT = nc.dram_tensor("sinmT", [Dh, S], f16, kind="ExternalInput").ap()
    maskadd = nc.dram_tensor("maskadd", [N_SB, 128, SB], f16,
                             kind="ExternalInput").ap()
    out = nc.dram_tensor("out", [S, D], f16, kind="ExternalOutput").ap()

    inv_sqrt_dh = 1.0 / math.sqrt(Dh)

    with tile.TileContext(nc) as tc, \
         tc.tile_pool(name="persist", bufs=1) as persist, \
         tc.tile_pool(name="xs", bufs=6) as xs, \
         tc.tile_pool(name="rope", bufs=3) as rope, \
         tc.tile_pool(name="vtp", bufs=1) as vtp, \
         tc.tile_pool(name="expp", bufs=4) as expp, \
         tc.tile_pool(name="zpool", bufs=2) as zpool, \
         tc.tile_pool(name="obuf", bufs=3) as obuf, \
         tc.tile_pool(name="proj_ps", bufs=1, space="PSUM") as proj_ps, \
         tc.tile_pool(name="hot_ps", bufs=3, space="PSUM") as hot_ps, \
         tc.tile_pool(name="accq_ps", bufs=1, space="PSUM") as accq_ps:

        # ---------------- persistent SBUF tensors --------------------------
        wq_t = [persist.tile([128, 8, EH], f16, tag=f"wq{i}", name=f"wq{i}")
                for i in range(4)]
        wk_t = [persist.tile([128, 16, Dh], f16, tag=f"wk{i}", name=f"wk{i}")
                for i in range(2)]
        wv_t = [persist.tile([128, 16, Dh], f16, tag=f"wv{i}", name=f"wv{i}")
                for i in range(2)]
        wo_r = persist.tile([128, G, D], f16, tag="wo_r")
        cos_sb = persist.tile([128, S], f16, tag="cos")
        sinm_sb = persist.tile([128, S], f16, tag="sinm")
        madd_sb = persist.tile([128, N_SB, SB], f16, tag="madd")
        ones_h = persist.tile([128, 128], f16, tag="ones")
        ident = persist.tile([128, 128], f16, tag="ident")
        ebias = persist.tile([128, 1], f32, tag="ebias")

        qr = [[persist.tile([128, SB], f16, tag=f"qr{h}_{sb}",
                            name=f"qr{h}_{sb}") for sb in range(N_SB)]
              for h in range(G)]
        kr = [persist.tile([128, SB], f16, tag=f"kr{sb}", name=f"kr{sb}")
              for sb in range(N_SB)]
        vnat = [persist.tile([128, N_SB, Dh], f16, tag=f"vnat{sb}",
                             name=f"vnat{sb}") for sb in range(N_SB)]
        attn = [[persist.tile([128, SB], f16, tag=f"attn{t}_{h}",
                              name=f"attn{t}_{h}") for h in range(G)]
                for t in range(N_SB)]

        nc.vector.memset(ones_h, 1.0)
        nc.vector.memset(ebias, -EXP_BIAS)
        make_identity(nc, ident)

        # ---------------- input DMAs (all on the SP queue) -----------------
        wq_re = wqT.rearrange("(c p) e -> p c e", p=128)
        wk_re = wkT.rearrange("(c p) e -> p c e", p=128)
        wv_re = wvT.rearrange("(c p) e -> p c e", p=128)
        # tiny first-chunk tiles so the first matmul starts ~15us earlier
        w00 = persist.tile([128, 1, EH], f16, tag="w00")
        x00 = persist.tile([128, 1, SB], f16, tag="x00")
        nc.sync.dma_start(out=w00, in_=wq_re[:, 0:1, :])
        nc.sync.dma_start(out=x00,
                          in_=xT[0:128, 0:SB].rearrange("(c p) s -> p c s",
                                                        p=128))

        def load_x_q(sb, q):
            lo = xT[q * 1024:(q + 1) * 1024,
                    sb * SB:(sb + 1) * SB].rearrange("(c p) s -> p c s",
                                                     p=128)
            ta = xs.tile([128, 8, SB], f16, tag="xh", name=f"x{sb}_{q}")
            nc.sync.dma_start(out=ta, in_=lo)
            return ta

        # sb0: stream weights and x in pass-A consumption order
        xtiles = {0: []}
        for q in range(4):
            nc.sync.dma_start(out=wq_t[q], in_=wq_re[:, q * 8:(q + 1) * 8, :])
            xtiles[0].append(load_x_q(0, q))
            if q == 1:
                nc.sync.dma_start(out=wk_t[0], in_=wk_re[:, 0:16, :])
                nc.sync.dma_start(out=wv_t[0], in_=wv_re[:, 0:16, :])
        nc.sync.dma_start(out=wk_t[1], in_=wk_re[:, 16:32, :])
        nc.sync.dma_start(out=wv_t[1], in_=wv_re[:, 16:32, :])
        nc.sync.dma_start(out=cos_sb, in_=cosT)
        nc.sync.dma_start(out=sinm_sb, in_=sinmT)
        nc.sync.dma_start(out=madd_sb, in_=maskadd.rearrange("j p q -> p j q"))
        xtiles[1] = [load_x_q(1, q) for q in range(4)]
        nc.sync.dma_start(out=wo_r, in_=woT.rearrange("(h p) d -> p h d",
                                                      p=128))

        def load_x(sb):
            return [load_x_q(sb, q) for q in range(4)]

        # ---------------- emission units -----------------------------------
        PASS_OUT = [(0, 1, 2), (3, 4, 5)]   # 0-3 = q heads, 4 = k, 5 = v
        deferred_vt = []

        def emit_transposes():
            while deferred_vt:
                sb_, vt = deferred_vt.pop(0)
                for jj in range(SB // 128):
                    tp = hot_ps.tile([128, 128], f16, tag="hot", name="tp")
                    nc.tensor.transpose(
                        tp[:], vt[:, jj * 128:(jj + 1) * 128], ident[:])
                    nc.scalar.copy(out=vnat[sb_][:, jj, :], in_=tp[:])

        def proj_pass(sb, p, acc_sl=None):
            """Generator: yields after each dc pair (6 matmuls)."""
            ss = slice(sb * SB, (sb + 1) * SB)
            if acc_sl is None:
                acc = proj_ps.tile([128, 3, SB], f32, tag="projacc",
                                   name=f"acc{sb}_{p}")
                acc_sl = [acc[:, j, :] for j in range(3)]
            outs = PASS_OUT[p]
            for dc in range(N_DC):
                if sb == 0 and dc < 1:
                    xf = x00[:, dc, :]
                else:
                    xf = xtiles[sb][dc // 8][:, dc % 8, :]
                st_flags = dict(start=(dc == 0), stop=(dc == N_DC - 1))
                for j, o in enumerate(outs):
                    if o < G:
                        if sb == 0 and p == 0 and dc < 1:
                            w = w00[:, dc, o * 128:(o + 1) * 128]
                        else:
                            w = wq_t[dc // 8][:, dc % 8, o * 128:(o + 1) * 128]
                    elif o == 4:
                        w = wk_t[dc // 16][:, dc % 16, :]
                    else:
                        w = wv_t[dc // 16][:, dc % 16, :]
                    nc.tensor.matmul(acc_sl[j], w, xf, **st_flags)
                if dc % 2 == 1:
                    yield
            # drains + RoPE (DVE reads PSUM directly, partition-shifted).
            # V is drained first (ACT, ready immediately); its PE-transposes
            # are DEFERRED so they don't head-of-line block the PE queue
            # behind the serial DVE rope chain.
            if 5 in outs:
                a = acc_sl[outs.index(5)]
                vt = vtp.tile([128, SB], f16, tag="vt", name="vt")
                nc.scalar.copy(out=vt, in_=a)
                deferred_vt.append((sb, vt))
            for j, o in enumerate(outs):
                a = acc_sl[j]
                if o == 5:
                    continue
                dst = qr[o][sb] if o < G else kr[sb]
                t_cos = rope.tile([128, SB], f16, tag="t_cos", name="t_cos")
                t_sw = rope.tile([128, SB], f16, tag="t_sw", name="t_sw")
                nc.vector.tensor_mul(t_cos, a, cos_sb[:, ss])
                nc.vector.tensor_mul(t_sw[0:64, :], a[64:128, :],
                                     sinm_sb[0:64, ss])
                nc.vector.tensor_mul(t_sw[64:128, :], a[0:64, :],
                                     sinm_sb[64:128, ss])
                nc.gpsimd.tensor_add(dst[:], t_cos[:], t_sw[:])
            yield

        # attention chunks, software-pipelined: part1 (score+mask+exp) runs
        # one chunk ahead of part2 (sum+pv matmuls + epilogue).
        attn_state = {}
        pending = []

        def attn_part1(t, h, c):
            sc = hot_ps.tile([128, SB], f32, tag="hot", name="sc")
            nc.tensor.matmul(sc[:], kr[c // 4][:, (c % 4) * 128:
                                               (c % 4 + 1) * 128],
                             qr[h][t][:], start=True, stop=True)
            j = c - 4 * t
            if j >= 0:  # chunk contains the causal diagonal
                nc.vector.tensor_add(sc[:], sc[:], madd_sb[:, j, :])
            e = expp.tile([128, SB], f16, tag="e", name="e")
            nc.scalar.activation(e[:], sc[:], AF.Exp, scale=inv_sqrt_dh,
                                 bias=ebias[:])
            return e

        def attn_part2(t, h, c, e):
            n = 4 * (t + 1)
            if (t, h) not in attn_state:
                attn_state[(t, h)] = accq_ps.tile(
                    [128, 2, SB], f32, tag="accq", name=f"accq{t}_{h}")
            acc = attn_state[(t, h)]
            mmf = dict(start=(c == 0), stop=(c == n - 1))
            nc.tensor.matmul(acc[:, 0, :], vnat[c // 4][:, c % 4, :], e[:],
                             **mmf)
            nc.tensor.matmul(acc[:, 1, :], ones_h[:], e[:], **mmf)
            if c == n - 1:
                acc = attn_state.pop((t, h))
                # Drain PSUM fast: pv copy (ACT) || 1/Z approx (DVE) free the
                # accq banks in ~0.9us; normalize SBUF-side afterwards.
                pvs = zpool.tile([128, SB], f32, tag="pvs", name="pvs")
                rbc = zpool.tile([128, SB], f32, tag="rbc", name="rbc")
                nc.scalar.copy(out=pvs, in_=acc[:, 0, :])
                nc.vector.reciprocal_approx_fast(out=rbc[:], in_=acc[:, 1, :])
                nc.vector.tensor_mul(attn[t][h][:], pvs[:], rbc[:])

        def attn_chunk(t, h, c):
            e = attn_part1(t, h, c)
            # depth-2 software pipeline: part2(c) issues after part1(c+2),
            # giving exp(c) two chunks of PE work to hide behind.
            if len(pending) >= 2:
                attn_part2(*pending.pop(0))
            pending.append((t, h, c, e))

        def flush_pending():
            while pending:
                attn_part2(*pending.pop(0))

        # outproj group: 4 matmuls + drain; DMA store every 2 groups
        ob_state = {}

        def outproj_group(t, st, nb):
            cs = slice(nb * SB, (nb + 1) * SB)
            op = hot_ps.tile([128, SB], f32, tag="hot", name="op")
            j = st - 4 * t
            for hh in range(G):
                nc.tensor.matmul(op[:], attn[t][hh][:, j * 128:(j + 1) * 128],
                                 wo_r[:, hh, cs], start=(hh == 0),
                                 stop=(hh == G - 1))
            if nb % 2 == 0:
                ob_state[st] = obuf.tile([128, 2, SB], f16, tag="ob",
                                         name="ob")
            ob = ob_state[st]
            if (nb // 2) % 2 == 0:
                nc.scalar.copy(out=ob[:, nb % 2, :], in_=op[:])
            else:
                nc.vector.tensor_copy(out=ob[:, nb % 2, :], in_=op[:])
            if nb % 2 == 1:
                rs = slice(st * 128, (st + 1) * 128)
                nc.sync.dma_start(
                    out=out[rs, (nb - 1) * SB:(nb + 1) * SB],
                    in_=ob_state.pop(st))

        # ---------------- master schedule -----------------------------------
        from collections import deque

        # sb0: projections only. Pass B borrows the (still idle) attention
        # PSUM banks so its matmuls don't wait for pass A's drain.
        for _ in proj_pass(0, 0):
            pass
        acc_b0 = accq_ps.tile([128, 2, SB], f32, tag="accq", name="accq_p0")
        acc_b1 = hot_ps.tile([128, SB], f32, tag="hot", name="hot_p0")
        for _ in proj_pass(0, 1, acc_sl=[acc_b0[:, 0, :], acc_b0[:, 1, :],
                                         acc_b1[:]]):
            pass

        BURST_A = 10  # attn chunks emitted right after a proj pass drain

        for sb in range(1, N_SB + 1):
            t = sb - 1
            aq = deque((t, h, c) for h in range(G)
                       for c in range(4 * (t + 1)))
            oq = deque()
            if t >= 1:
                oq.extend((t - 1, st, nb)
                          for st in range(4 * (t - 1), 4 * (t - 1) + 4)
                          for nb in range(N_NB))

            def take_a(k):
                for _ in range(min(k, len(aq))):
                    attn_chunk(*aq.popleft())

            def take_o(k):
                for _ in range(min(k, len(oq))):
                    outproj_group(*oq.popleft())

            emit_transposes()
            if sb >= 2:
                # head-start: this step's attn units are already runnable;
                # put a few ahead of the proj pass (which waits on the
                # previous rope drain) so the PE queue never goes dry.
                take_a(4)
            if sb < N_SB:
                if sb + 1 < N_SB:
                    xtiles[sb + 1] = load_x(sb + 1)
                # spread attn/op units over the 32 dc-pair slots, keeping
                # BURST_A attn chunks in reserve for each pass boundary
                na = max(len(aq) - 2 * BURST_A, 0)
                no_res = max(len(oq) - 8, 0)
                no = len(oq)
                ad = od = 0
                for p in range(2):
                    g = proj_pass(sb, p)
                    for k in range(16):
                        next(g)
                        slot = p * 16 + k + 1
                        wa = na * slot // 32
                        wo = no_res * slot // 32
                        take_a(wa - ad)
                        take_o(wo - od)
                        ad, od = wa, od + min(wo - od, no - od)
                    next(g)          # drains + rope emission
                    take_o(4)        # pure-PE filler for the drain bubble
                    take_a(BURST_A)
                    emit_transposes()
                    for _ in g:      # exhaust (no-op)
                        pass
            # tail: whatever is left (also covers the next step's start)
            while aq or oq:
                take_a(2)
                take_o(1)
            flush_pending()
        # final outproj for t=3
        for st in range(12, 16):
            for nb in range(N_NB):
                outproj_group(3, st, nb)

    nc.compile()
    return nc


def _prep_inputs(hidden_states, Wq, Wk, Wv, Wo, cos, sin):
    x = np.asarray(hidden_states, dtype=np.float32).reshape(S, D)
    Wq = np.asarray(Wq, dtype=np.float32)
    Wk = np.asarray(Wk, dtype=np.float32)
    Wv = np.asarray(Wv, dtype=np.float32)
    Wo = np.asarray(Wo, dtype=np.float32)
    cos = np.asarray(cos, dtype=np.float32)
    sin = np.asarray(sin, dtype=np.float32)

    xT = np.ascontiguousarray(x.T).astype(np.float16)
    cosT = np.ascontiguousarray(cos.T).astype(np.float16)
    sinmT = np.ascontiguousarray(sin.T).copy()
    sinmT[: Dh // 2] *= -1.0
    sinmT = sinmT.astype(np.float16)
    maskadd = np.zeros((N_SB, 128, SB), dtype=np.float32)
    kp = np.arange(128)[:, None]
    qc = np.arange(SB)[None, :]
    for j in range(N_SB):
        maskadd[j] = np.where(kp + 128 * j > qc, -1e4, 0.0)
    maskadd = maskadd.astype(np.float16)

    in_maps = []
    for c in range(N_CORES):
        in_maps.append({
            "xT": xT,
            "wqT": np.ascontiguousarray(
                Wq[c * EH:(c + 1) * EH, :].T).astype(np.float16),
            "wkT": np.ascontiguousarray(
                Wk[c * Dh:(c + 1) * Dh, :].T).astype(np.float16),
            "wvT": np.ascontiguousarray(
                Wv[c * Dh:(c + 1) * Dh, :].T).astype(np.float16),
            "woT": np.ascontiguousarray(
                Wo[:, c * EH:(c + 1) * EH].T).astype(np.float16),
            "cosT": cosT,
            "sinmT": sinmT,
            "maskadd": maskadd,
        })
    return in_maps


def run(trace=False, **inputs):
    """Run on hardware; returns (full_output, exec_time_ns or None)."""
    from concourse.bass_utils import run_bass_kernel_spmd

    if trace:
        _install_ntff_hook()
    if "nc" not in _cache:
        _cache["nc"] = _build()
    nc = _cache["nc"]
    in_maps = _prep_inputs(**inputs)
    res = run_bass_kernel_spmd(nc, in_maps, core_ids=list(range(N_CORES)),
                               trace=trace)
    acc = res.results[0]["out"].astype(np.float32)
    for c in range(1, N_CORES):
        acc += res.results[c]["out"].astype(np.float32)
    return acc.reshape(B, S, D), res.exec_time_ns


def _install_ntff_hook():
    """Register the axon NTFF profiling hook missing from this image."""
    import types
    try:
        import antenv
        from trn_agent_boot.trn_boot import _ntff_profile_via_ctypes
    except ImportError:
        return
    if "antenv.axon_hooks" in sys.modules:
        return
    mod = types.ModuleType("antenv.axon_hooks")
    mod._hook = _ntff_profile_via_ctypes("/opt/axon/libaxon_pjrt.so")
    mod.get_axon_ntff_profile_hook = lambda: mod._hook
    mod.set_axon_ntff_profile_hook = lambda h: setattr(mod, "_hook", h)
    sys.modules["antenv.axon_hooks"] = mod
    antenv.axon_hooks = mod


def kernel(**inputs):
    out, _ = run(trace=False, **inputs)
    return out
